# revision 29
# baseline (speedup 1.0000x reference)
"""Trainium2 Bass kernel for BasisDecorrelationLoss.

Math: per sample b, with x = depth_basis[b] ([C=32, N=76800]) and mask m ([N]):
    mu_c  = (1/N) sum_n x[c,n]                      (unmasked spatial mean)
    S_cd  = sum_n x[c,n] x[d,n] m[n]                (masked Gram, the heavy part)
    t_c   = sum_n x[c,n] m[n]
    M     = sum_n m[n]
    cov   = (S - mu t^T - t mu^T + mu mu^T M) / M   (mean-centered masked covariance)
    zncc  = clamp(cov,eps) / (sigma sigma^T), loss_b = mean(zncc^2)
    loss  = mean_b loss_b

Device strategy (data-parallel, one sample per NeuronCore, 8 cores):
  The host concatenates x and m into one [33, N] tensor per sample. SBUF uses a
  "slab" layout: partition p holds n in [p*600, (p+1)*600) as 33 strips of
  contiguous floats (contiguous DMA runs, no transposes, one DMA per chunk).
  One augmented matmul accumulation computes S, t, sum_x and M at once: per
  contraction step j, weights lhsT = [X_j*m | 1] (33 cols, strided AP) and
  moving rhs = [X_j | m_j | 1] (34 cols) accumulate into PSUM [33, 34] over 600
  K=128 steps:
      out[c<32, d<32] = S,  out[c<32, 33] = t,  out[32, d<32] = N*mu,
      out[32, 32] = M,      out[32, 33] = N.
  The mask multiply is one DVE tensor_tensor per chunk with a stride-0
  broadcast AP. Host does the final [32,32] math and averages the 8 per-sample
  scalars (the "scalar all-reduce").
"""

import ml_dtypes
import numpy as np

import concourse.bacc as bacc
import concourse.bass as bass
import concourse.tile as tile
import concourse.tile_rust as tile_rust
from concourse import mybir
from concourse.bass_utils import run_bass_kernel_spmd

B = 8
C = 32
H, W = 240, 320
N = H * W            # 76800
P = 128              # SBUF partitions
NPP = N // P         # 600 n-values per partition
CHUNKS = [120, 240, 240]   # j-extents; first is small so the PE starts early
NQ = len(CHUNKS)
EPS = 1e-10

_F32 = mybir.dt.float32
_BF16 = mybir.dt.bfloat16


NG = 3               # col-groups used for the Gram (j mod NG)
JS = 30              # j-extent per compute sub-split


def _build_kernel_body(tc: "tile.TileContext", xm_d: bass.AP, out_d: bass.AP):
    nc = tc.nc

    # n = p*NPP + u ; u chunked per CHUNKS; rows 0..31 = x, row 32 = m
    xm_flat = xm_d.rearrange("c (p u) -> p c u", p=P)

    with (
        tc.tile_pool(name="slabs", bufs=2) as slabs,
        tc.tile_pool(name="psum", bufs=1, space="PSUM") as psum,
        tc.tile_pool(name="outp", bufs=1) as outp,
    ):
        # NG blocks of [32, 34]: block g accumulates Gram+t over j = g (mod NG)
        acc = psum.tile([NG * C, C + 2], _F32)
        junk_full = psum.tile([P, 1], _F32)
        junk = junk_full[96:97, :]

        off = 0
        for q, JC in enumerate(CHUNKS):
            # bf16 stream slab straight from HBM:
            # strips 0..31 = x, 32 = m (one DMA), 33 = ones (memset)
            s_t = slabs.tile([P, C + 2, JC], _BF16, tag="s_t")
            nc.sync.dma_start(out=s_t[:, 0 : C + 1, :],
                              in_=xm_flat[:, :, off : off + JC])
            nc.vector.memset(s_t[:, C + 1, :], 1.0)
            # bf16 weights slab: 32 strips of x*m (mask broadcast over strips)
            w_t = slabs.tile([P, C, JC], _BF16, tag="w_t")

            # Sync-carrier matmul: walrus gives LDWEIGHTS a single sync-wait
            # slot, but the first Gram matmul of a chunk would need two (DMA
            # for s_t + DVE for w_t). This 1x1 matmul reads only DMA-written
            # strips, consuming the DMA wait on PE first.
            carrier = nc.tensor.matmul(
                junk[:, :],
                lhsT=s_t[:, C, 0:1],
                rhs=s_t[:, C, 0:1],
                start=True,
                stop=True,
                tile_position=(0, 96),
            )

            # mask-multiply -> matmuls in j-subchunks so the PE can start
            # after only JS steps of the chunk are multiplied
            for s in range(JC // JS):
                jsl = slice(s * JS, (s + 1) * JS)
                m_strip = s_t[:, C, jsl]
                m_bc = bass.AP(
                    tensor=m_strip.tensor,
                    offset=m_strip.offset,
                    ap=[m_strip.ap[0], [0, C], m_strip.ap[1]],
                )
                nc.vector.tensor_mul(w_t[:, :, jsl], s_t[:, 0:C, jsl], m_bc)

                first_mm = None
                for j in range(s * JS, (s + 1) * JS):
                    jg = off + j
                    g = jg % NG
                    mm = nc.tensor.matmul(
                        acc[32 * g : 32 * (g + 1), :],
                        lhsT=w_t[:, :, j],
                        rhs=s_t[:, :, j],
                        start=(jg < NG),
                        stop=(jg >= NPP - NG),
                        tile_position=(0, 32 * g),
                    )
                    if first_mm is None:
                        first_mm = mm
                        tile_rust.add_dep_helper(
                            mm.ins, carrier.ins, sync=False,
                            reason="carrier drains the DMA wait first",
                        )
            off += JC

        res = outp.tile([NG * C, C + 2], _F32)
        nc.any.tensor_copy(res, acc)
        nc.sync.dma_start(out=out_d, in_=res)


def _build_nc() -> bass.Bass:
    nc = bacc.Bacc()
    xm = nc.declare_dram_parameter("xm", [C + 1, N], _BF16, isOutput=False)
    out = nc.declare_dram_parameter("out", [NG * C, C + 2], _F32,
                                    isOutput=True)
    with tile.TileContext(nc) as tc:
        _build_kernel_body(tc, xm[:], out[:])
    nc.finalize()
    return nc


def _finalize(gathered: list[np.ndarray],
              host_stats: np.ndarray) -> np.ndarray:
    """Host-side per-sample [96, 34] Gram blocks -> scalar loss, batch mean.

    host_stats[i] = [sum_n x_c (c=0..31), sum_n m] for sample i, computed on
    the host from the bf16-rounded input (matching what the device sees).
    """
    total = 0.0
    for i, G in enumerate(gathered):
        G = G.astype(np.float64)
        S = np.zeros((C, C))
        t = np.zeros(C)
        for g in range(NG):
            S += G[32 * g : 32 * (g + 1), 0:C]
            t += G[32 * g : 32 * (g + 1), C + 1]
        stats = host_stats[i]
        mu = stats[0:C] / N
        M = stats[C]
        cov = (S - np.outer(mu, t) - np.outer(t, mu) + np.outer(mu, mu) * M) / M
        cov = np.maximum(cov, EPS)
        sig = np.sqrt(np.diag(cov))
        zncc = cov / np.outer(sig, sig)
        total += float(np.mean(zncc * zncc))
    return np.array(total / B, dtype=np.float32)


_NC_CACHE = None


def _run(depth_basis: np.ndarray, mask: np.ndarray, trace: bool = False):
    global _NC_CACHE
    if _NC_CACHE is None:
        _NC_CACHE = _build_nc()
    nc = _NC_CACHE

    x_full = np.asarray(depth_basis, dtype=np.float32).reshape(B, C, N)
    m_full = np.asarray(mask, dtype=np.float32).reshape(B, 1, N)
    xm_full = np.concatenate([x_full, m_full], axis=1).astype(
        ml_dtypes.bfloat16)

    host_stats = xm_full.astype(np.float64).sum(axis=2)  # [B, 33]

    in_maps = [{"xm": xm_full[i]} for i in range(B)]
    r = run_bass_kernel_spmd(nc, in_maps, list(range(B)), trace=trace)
    gathered = [np.asarray(r.results[i]["out"]) for i in range(B)]
    return _finalize(gathered, host_stats), r


def kernel(depth_basis: np.ndarray, mask: np.ndarray) -> np.ndarray:
    loss, _ = _run(depth_basis, mask, trace=False)
    return loss


# revision 30
# speedup vs baseline: 1.0911x; 1.0911x over previous
"""Trainium2 Bass kernel for BasisDecorrelationLoss.

Math: per sample b, with x = depth_basis[b] ([C=32, N=76800]) and mask m ([N]):
    mu_c  = (1/N) sum_n x[c,n]                      (unmasked spatial mean)
    S_cd  = sum_n x[c,n] x[d,n] m[n]                (masked Gram, the heavy part)
    t_c   = sum_n x[c,n] m[n]
    M     = sum_n m[n]
    cov   = (S - mu t^T - t mu^T + mu mu^T M) / M   (mean-centered masked covariance)
    zncc  = clamp(cov,eps) / (sigma sigma^T), loss_b = mean(zncc^2)
    loss  = mean_b loss_b

Device strategy (data-parallel, one sample per NeuronCore, 8 cores):
  The host concatenates x and m into one [33, N] tensor per sample. SBUF uses a
  "slab" layout: partition p holds n in [p*600, (p+1)*600) as 33 strips of
  contiguous floats (contiguous DMA runs, no transposes, one DMA per chunk).
  One augmented matmul accumulation computes S, t, sum_x and M at once: per
  contraction step j, weights lhsT = [X_j*m | 1] (33 cols, strided AP) and
  moving rhs = [X_j | m_j | 1] (34 cols) accumulate into PSUM [33, 34] over 600
  K=128 steps:
      out[c<32, d<32] = S,  out[c<32, 33] = t,  out[32, d<32] = N*mu,
      out[32, 32] = M,      out[32, 33] = N.
  The mask multiply is one DVE tensor_tensor per chunk with a stride-0
  broadcast AP. Host does the final [32,32] math and averages the 8 per-sample
  scalars (the "scalar all-reduce").
"""

import ml_dtypes
import numpy as np

import concourse.bacc as bacc
import concourse.bass as bass
import concourse.tile as tile
import concourse.tile_rust as tile_rust
from concourse import mybir
from concourse.bass_utils import run_bass_kernel_spmd

B = 8
C = 32
H, W = 240, 320
N = H * W            # 76800
P = 128              # SBUF partitions
NPP = N // P         # 600 n-values per partition
CHUNKS = [300, 300]  # j-extents per chunk (600B DMA runs)
NQ = len(CHUNKS)
EPS = 1e-10

_F32 = mybir.dt.float32
_BF16 = mybir.dt.bfloat16


NG = 3               # col-groups used for the Gram (j mod NG)
JS = 30              # j-extent per compute sub-split


def _build_kernel_body(tc: "tile.TileContext", xm_d: bass.AP, out_d: bass.AP):
    nc = tc.nc

    # n = p*NPP + u ; u chunked per CHUNKS; rows 0..31 = x, row 32 = m
    xm_flat = xm_d.rearrange("c (p u) -> p c u", p=P)

    with (
        tc.tile_pool(name="slabs", bufs=2) as slabs,
        tc.tile_pool(name="psum", bufs=1, space="PSUM") as psum,
        tc.tile_pool(name="outp", bufs=1) as outp,
    ):
        # NG blocks of [32, 34]: block g accumulates Gram+t over j = g (mod NG)
        acc = psum.tile([NG * C, C + 2], _F32)
        junk_full = psum.tile([P, 1], _F32)
        junk = junk_full[96:97, :]

        off = 0
        for q, JC in enumerate(CHUNKS):
            # bf16 stream slab straight from HBM:
            # strips 0..31 = x, 32 = m (one DMA), 33 = ones (memset)
            s_t = slabs.tile([P, C + 2, JC], _BF16, tag="s_t")
            nc.sync.dma_start(out=s_t[:, 0 : C + 1, :],
                              in_=xm_flat[:, :, off : off + JC])
            nc.vector.memset(s_t[:, C + 1, :], 1.0)
            # bf16 weights slab: 32 strips of x*m (mask broadcast over strips)
            w_t = slabs.tile([P, C, JC], _BF16, tag="w_t")

            # Sync-carrier matmul: walrus gives LDWEIGHTS a single sync-wait
            # slot, but the first Gram matmul of a chunk would need two (DMA
            # for s_t + DVE for w_t). This 1x1 matmul reads only DMA-written
            # strips, consuming the DMA wait on PE first.
            carrier = nc.tensor.matmul(
                junk[:, :],
                lhsT=s_t[:, C, 0:1],
                rhs=s_t[:, C, 0:1],
                start=True,
                stop=True,
                tile_position=(0, 96),
            )

            # mask-multiply -> matmuls in j-subchunks so the PE can start
            # after only JS steps of the chunk are multiplied
            for s in range(JC // JS):
                jsl = slice(s * JS, (s + 1) * JS)
                m_strip = s_t[:, C, jsl]
                m_bc = bass.AP(
                    tensor=m_strip.tensor,
                    offset=m_strip.offset,
                    ap=[m_strip.ap[0], [0, C], m_strip.ap[1]],
                )
                nc.vector.tensor_mul(w_t[:, :, jsl], s_t[:, 0:C, jsl], m_bc)

                first_mm = None
                for j in range(s * JS, (s + 1) * JS):
                    jg = off + j
                    g = jg % NG
                    mm = nc.tensor.matmul(
                        acc[32 * g : 32 * (g + 1), :],
                        lhsT=w_t[:, :, j],
                        rhs=s_t[:, :, j],
                        start=(jg < NG),
                        stop=(jg >= NPP - NG),
                        tile_position=(0, 32 * g),
                    )
                    if first_mm is None:
                        first_mm = mm
                        tile_rust.add_dep_helper(
                            mm.ins, carrier.ins, sync=False,
                            reason="carrier drains the DMA wait first",
                        )
            off += JC

        res = outp.tile([NG * C, C + 2], _F32)
        nc.any.tensor_copy(res, acc)
        nc.sync.dma_start(out=out_d, in_=res)


def _build_nc() -> bass.Bass:
    nc = bacc.Bacc()
    xm = nc.declare_dram_parameter("xm", [C + 1, N], _BF16, isOutput=False)
    out = nc.declare_dram_parameter("out", [NG * C, C + 2], _F32,
                                    isOutput=True)
    with tile.TileContext(nc) as tc:
        _build_kernel_body(tc, xm[:], out[:])
    nc.finalize()
    return nc


def _finalize(gathered: list[np.ndarray],
              host_stats: np.ndarray) -> np.ndarray:
    """Host-side per-sample [96, 34] Gram blocks -> scalar loss, batch mean.

    host_stats[i] = [sum_n x_c (c=0..31), sum_n m] for sample i, computed on
    the host from the bf16-rounded input (matching what the device sees).
    """
    total = 0.0
    for i, G in enumerate(gathered):
        G = G.astype(np.float64)
        S = np.zeros((C, C))
        t = np.zeros(C)
        for g in range(NG):
            S += G[32 * g : 32 * (g + 1), 0:C]
            t += G[32 * g : 32 * (g + 1), C + 1]
        stats = host_stats[i]
        mu = stats[0:C] / N
        M = stats[C]
        cov = (S - np.outer(mu, t) - np.outer(t, mu) + np.outer(mu, mu) * M) / M
        cov = np.maximum(cov, EPS)
        sig = np.sqrt(np.diag(cov))
        zncc = cov / np.outer(sig, sig)
        total += float(np.mean(zncc * zncc))
    return np.array(total / B, dtype=np.float32)


_NC_CACHE = None


def _run(depth_basis: np.ndarray, mask: np.ndarray, trace: bool = False):
    global _NC_CACHE
    if _NC_CACHE is None:
        _NC_CACHE = _build_nc()
    nc = _NC_CACHE

    x_full = np.asarray(depth_basis, dtype=np.float32).reshape(B, C, N)
    m_full = np.asarray(mask, dtype=np.float32).reshape(B, 1, N)
    xm_full = np.concatenate([x_full, m_full], axis=1).astype(
        ml_dtypes.bfloat16)

    host_stats = xm_full.astype(np.float64).sum(axis=2)  # [B, 33]

    in_maps = [{"xm": xm_full[i]} for i in range(B)]
    r = run_bass_kernel_spmd(nc, in_maps, list(range(B)), trace=trace)
    gathered = [np.asarray(r.results[i]["out"]) for i in range(B)]
    return _finalize(gathered, host_stats), r


def kernel(depth_basis: np.ndarray, mask: np.ndarray) -> np.ndarray:
    loss, _ = _run(depth_basis, mask, trace=False)
    return loss
